# revision 29
# baseline (speedup 1.0000x reference)
"""Linformer multi-head attention on 8 Trainium2 NeuronCores.

Sharding: data-parallel over batch (BATCH=8 -> 1 batch element per core).
Each core runs the full per-batch computation:
  q = x@wq, k = x@wk, v = x@wv            (per head h: 64-dim slices)
  k_proj[h] = E[h].T @ k[h]   [256, 64]   (contraction over seq)
  v_proj[h] = F[h].T @ v[h]   [256, 64]
  scores = q @ k_proj.T / 8   [4096, 256]
  attn = softmax(scores)  ;  out = attn @ v_proj
  y = concat_heads(out) @ w_out + b_out

v2 design notes (vs v1 at 506us):
  - x is transposed on HOST -> xT [512, 4096]; no on-chip transposes.
  - E/F are relayouted on HOST to [32 tiles, 128, 8 heads, 256] so each
    (j, s) DMA is one fully-contiguous 512KB block.
  - k_projT/v_projT accumulate in 4 persistent PSUM banks across all 32
    seq-tiles (no DVE partial adds).  A zero-matmul initializes each bank
    (has_written set everywhere) so every real matmul uses start=False --
    avoids the bank-wide has_written clear racing between interleaved
    accumulation regions.
  - M=64 kp/vp matmuls and K=64 score matmuls run as tile_position pairs
    (col/row-group concurrency, ~2x).
  - softmax denominator comes free from the PV matmul via an appended
    ones-column (row 64); per head-PAIR the two PV outputs land in one
    [128, 1024] PSUM tile so one reciprocal_approx_fast [1, 1024] handles
    both heads (v1 used full-precision reciprocal: 3.3us/op, 213us total).
  - reciprocal -> broadcast via rank-1 PE matmuls in float32r (full rate
    at N=512; plain f32 matmul is 4x slower).
  - evacuation work split between ScalarE (qt, bc) and VectorE (kv, oT
    muls, fin bias-adds); exp on ScalarE in [128, 1024] ops.

Compute dtype is bf16 (inputs cast on host) with fp32 PSUM accumulation.
"""

import os

import numpy as np
import ml_dtypes

BATCH, SEQ, DM = 8, 4096, 512
NH, DH, R = 8, 64, 256
NCORES = 8
NT = SEQ // 512  # 8 big n-tiles of 512 rows

_built = {}


def _build():
    """Build the Bass module (once per process)."""
    if "nc" in _built:
        return _built["nc"]

    from contextlib import ExitStack

    import concourse.bass as bass
    import concourse.bacc as bacc
    import concourse.mybir as mybir
    import concourse.tile as tile
    from concourse.masks import make_identity

    f32 = mybir.dt.float32
    f32r = mybir.dt.float32r
    cdt = mybir.dt.bfloat16

    nc = bacc.Bacc("TRN2", target_bir_lowering=False, debug=False)

    # xT: host-transposed [DM, SEQ]
    x_d = nc.dram_tensor("x", [DM, SEQ], cdt, kind="ExternalInput").ap()
    wq_d = nc.dram_tensor("wq", [DM, DM], cdt, kind="ExternalInput").ap()
    wk_d = nc.dram_tensor("wk", [DM, DM], cdt, kind="ExternalInput").ap()
    wv_d = nc.dram_tensor("wv", [DM, DM], cdt, kind="ExternalInput").ap()
    # E/F host layout: [ti, p, h, r] with ti = j*4+s, seq = ti*128+p
    e_d = nc.dram_tensor("E", [SEQ // 128, 128, NH, R], cdt, kind="ExternalInput").ap()
    f_d = nc.dram_tensor("F", [SEQ // 128, 128, NH, R], cdt, kind="ExternalInput").ap()
    wo_d = nc.dram_tensor("w_out", [DM, DM], cdt, kind="ExternalInput").ap()
    b_d = nc.dram_tensor("b_out", [DM], f32, kind="ExternalInput").ap()
    y_d = nc.dram_tensor("y", [SEQ, DM], f32, kind="ExternalOutput").ap()
    debug = os.environ.get("LINF_DEBUG", "0") == "1"
    if debug:
        dbg_d = nc.dram_tensor("dbg", [1, 4096], f32, kind="ExternalOutput").ap()

    with tile.TileContext(nc) as tc, ExitStack() as ctx:
        singles = ctx.enter_context(tc.tile_pool(name="singles", bufs=1))

        ident = singles.tile([128, 128], cdt)
        make_identity(nc, ident)
        ones_blk = singles.tile([128, 64], cdt)
        nc.vector.memset(ones_blk, 1.0)
        zeros128 = singles.tile([128, 128], cdt)
        nc.vector.memset(zeros128, 0.0)

        def act_recip(out, in_):
            """ACT Reciprocal LUT (bass blocks it for accuracy; softmax
            denominators only need ~1e-2 so the LUT is fine here)."""
            eng = nc.scalar
            ins = [eng.lower_ap(in_)]
            for val in (0.0, 1.0, 0.0):  # bias, scale, alpha
                ins.append(mybir.ImmediateValue(dtype=f32, value=val))
            return eng.add_instruction(
                mybir.InstActivation(
                    name=nc.get_next_instruction_name(),
                    func=mybir.ActivationFunctionType.Reciprocal,
                    ins=ins,
                    outs=[eng.lower_ap(out)],
                )
            )
        # bias replicated [128, 2, 512] for the [128, 1024] fin bias-add
        bias_bc = singles.tile([128, 2, DM], f32)
        b_bc_ap = bass.AP(
            tensor=b_d.tensor,
            offset=b_d.offset,
            ap=[[0, 128], [0, 2]] + list(b_d.ap),
        )
        nc.sync.dma_start(out=bias_bc, in_=b_bc_ap)

        # weights as [128, dk, 512]: chunk dk holds rows dk*128..+128
        w_sb = {}
        for name, d in (("wq", wq_d), ("wk", wk_d), ("wv", wv_d), ("wo", wo_d)):
            t = singles.tile([128, 4, DM], cdt, name=f"w_{name}")
            nc.sync.dma_start(out=t, in_=d.rearrange("(dk p) m -> p dk m", p=128))
            w_sb[name] = t

        # QT global [512, 4096] as 4 tiles [128, 4096]; tile t = heads 2t,2t+1
        qt_g = [singles.tile([128, SEQ], cdt, tag=f"qt{t}", name=f"qt{t}") for t in range(4)]
        # per-head low-rank projections, transposed [64, 256], packed 4/tile:
        # head h -> tile t=h//4, partition half ph=h%2, col half ch=(h//2)%2
        kpT_sb = [singles.tile([128, 2 * R], cdt, tag=f"kp{t}", name=f"kpT{t}") for t in range(2)]
        vpT_sb = [singles.tile([128, 2 * R], cdt, tag=f"vp{t}", name=f"vpT{t}") for t in range(2)]

        def hslice(sb, h):
            """[64, 256] slice of packed kpT/vpT for head h."""
            t, ph, ch = h // 4, h % 2, (h // 2) % 2
            return sb[t][ph * 64 : (ph + 1) * 64, ch * R : (ch + 1) * R]

        # v_proj natural chunks: [128, 2, 64] per head
        vext = singles.tile([128, NH, 2, 64], cdt)

        # ---------------- Phase AB: QT, k_projT, v_projT ----------------
        with (
            tc.tile_pool(name="p_x", bufs=3) as p_x,
            tc.tile_pool(name="p_ef", bufs=3) as p_ef,
            tc.tile_pool(name="p_kv", bufs=6) as p_kv,
            tc.tile_pool(name="ps_acc", bufs=1, space="PSUM") as ps_acc,
            tc.tile_pool(name="ps_mm", bufs=4, space="PSUM") as ps_mm,
        ):
            # persistent PSUM accumulators: 4 banks, live all of phase AB
            kpT_ps = [
                ps_acc.tile([128, 2 * R], f32, tag=f"kpp{t}", name=f"kpT_ps{t}")
                for t in range(2)
            ]
            vpT_ps = [
                ps_acc.tile([128, 2 * R], f32, tag=f"vpp{t}", name=f"vpT_ps{t}")
                for t in range(2)
            ]
            # init: one full-bank zero-matmul (lhsT=0 so rhs content is
            # irrelevant) sets has_written on every element so all real
            # accumulation matmuls can use start=False (see header).
            for t in range(2):
                for acc in (kpT_ps[t], vpT_ps[t]):
                    nc.tensor.matmul(
                        acc,
                        zeros128,
                        w_sb["wq"][:, 0, :],
                        start=True,
                        stop=False,
                        skip_group_check=True,
                    )

            x_r = x_d.rearrange("(dk p) n -> p dk n", p=128)
            e_r = e_d.rearrange("t p h r -> p t h r")
            f_r = f_d.rearrange("t p h r -> p t h r")
            for j in range(NT):  # 8 n-tiles of 512 rows
                xt = p_x.tile([128, 4, 512], cdt, tag="xt", name=f"xt_{j}")
                nc.sync.dma_start(out=xt, in_=x_r[:, :, j * 512 : (j + 1) * 512])

                e_t = p_ef.tile([128, 4, NH, R], cdt, tag="ef_e", name=f"e_{j}")
                f_t = p_ef.tile([128, 4, NH, R], cdt, tag="ef_f", name=f"f_{j}")
                nc.sync.dma_start(out=e_t, in_=e_r[:, j * 4 : (j + 1) * 4, :, :])
                nc.sync.dma_start(out=f_t, in_=f_r[:, j * 4 : (j + 1) * 4, :, :])
                e_ts = [e_t[:, s, :, :] for s in range(4)]
                f_ts = [f_t[:, s, :, :] for s in range(4)]

                # QT_j[dq] [128, 512] = sum_dk wq[dk, dq-chunk].T-form @ xT[dk]
                for dq in range(4):
                    pq = ps_mm.tile([128, 512], f32, tag="pmm", name=f"pq_{j}_{dq}")
                    for dk in range(4):
                        nc.tensor.matmul(
                            pq,
                            w_sb["wq"][:, dk, dq * 128 : (dq + 1) * 128],
                            xt[:, dk, :],
                            start=(dk == 0),
                            stop=(dk == 3),
                        )
                    nc.scalar.copy(qt_g[dq][:, j * 512 : (j + 1) * 512], pq)

                # K/V per 128-row subtile s, then accumulate projections
                for s in range(4):
                    for wname, ef in (("wk", e_ts[s]), ("wv", f_ts[s])):
                        pk = ps_mm.tile([128, 512], f32, tag="pmm", name=f"pk_{j}_{s}")
                        for dk in range(4):
                            nc.tensor.matmul(
                                pk,
                                xt[:, dk, s * 128 : (s + 1) * 128],
                                w_sb[wname][:, dk, :],
                                start=(dk == 0),
                                stop=(dk == 3),
                            )
                        kv_sb = p_kv.tile([128, 512], cdt, tag="kv", name=f"kv_{j}_{s}")
                        nc.vector.tensor_copy(kv_sb, pk)

                        acc = kpT_ps if wname == "wk" else vpT_ps
                        last = (j == NT - 1) and (s == 3)
                        # col-tiled pairs: heads (2i, 2i+1) -> partition
                        # halves 0/64 of the same bank, concurrent on PE.
                        for h in range(NH):
                            t, ph, ch = h // 4, h % 2, (h // 2) % 2
                            nc.tensor.matmul(
                                acc[t][
                                    ph * 64 : (ph + 1) * 64,
                                    ch * R : (ch + 1) * R,
                                ],
                                kv_sb[:, h * 64 : (h + 1) * 64],
                                ef[:, h, :],
                                start=False,
                                stop=last,
                                skip_group_check=True,
                            )

            for t in range(2):
                nc.scalar.copy(kpT_sb[t], kpT_ps[t])
                nc.scalar.copy(vpT_sb[t], vpT_ps[t])

        # ---------------- Phase C: attention + output dense ----------------
        y_r = y_d.rearrange("(t p) m -> p t m", p=128)  # t = j*4+s
        with (
            tc.tile_pool(name="p_at", bufs=10) as p_at,
            tc.tile_pool(name="p_bc", bufs=6) as p_bc,
            tc.tile_pool(name="p_ot", bufs=8) as p_ot,
            tc.tile_pool(name="p_fin", bufs=4) as p_fin,
            tc.tile_pool(name="ps_c", bufs=2, space="PSUM") as ps_c,
        ):
            # build vext: transpose v_projT[h] chunks to natural
            for h in range(NH):
                pv = ps_c.tile([128, 512], cdt, tag="op", bufs=2, name="pv")
                for rc in range(2):
                    nc.tensor.transpose(
                        pv[:, rc * 64 : (rc + 1) * 64],
                        hslice(vpT_sb, h)[:, rc * 128 : (rc + 1) * 128],
                        ident[(h % 2) * 64 : (h % 2) * 64 + 64, (h % 2) * 64 : (h % 2) * 64 + 64],
                    )
                for rc in range(2):
                    nc.vector.tensor_copy(
                        vext[:, h, rc, :], pv[:, rc * 64 : (rc + 1) * 64]
                    )

            for j in range(NT):
                oT = [p_ot.tile([128, 512], cdt, tag="ot", name=f"oT{j}_{t}") for t in range(4)]
                # all 8 heads' scores+exp first so the ACT stream runs the 8
                # Exp ops back-to-back, then the 4 Reciprocal ops: 2 ACT
                # table switches per j instead of 8 (each reload is 1.3us).
                at_j = []
                for h in range(NH):
                    ph = h % 2
                    qrow = qt_g[h // 2][
                        ph * 64 : ph * 64 + 64, j * 512 : (j + 1) * 512
                    ]
                    # scoresT [256, 512] as one [128, 1024] tile
                    # (rc chunks in col halves); K=64 row-tiled pair
                    # with the other head of hp runs concurrently.
                    sc = ps_c.tile([128, 1024], f32, tag="sc", name=f"sc{j}_{h}")
                    for rc in range(2):
                        nc.tensor.matmul(
                            sc[:, rc * 512 : (rc + 1) * 512],
                            hslice(kpT_sb, h)[:, rc * 128 : (rc + 1) * 128],
                            qrow,
                            start=True,
                            stop=True,
                        )
                    a = p_at.tile([128, 1024], cdt, tag="at", name=f"at{j}_{h}")
                    nc.scalar.activation(
                        a, sc, mybir.ActivationFunctionType.Exp, scale=0.125
                    )
                    at_j.append(a)

                # pass 1: denominators for all 4 pairs (needs only at_j) so
                # the ACT reciprocals run right after the exps -- keeps the
                # ACT stream [exp x8, recip x4] per j (2 table switches).
                recs = []
                for hp in range(4):
                    ats = at_j[hp * 2 : hp * 2 + 2]
                    # den broadcast via all-ones stationary matmuls over
                    # attnT: rows 0..63 = den_h0, rows 64..127 = den_h1
                    # (col-tiled concurrent).  A zero-matmul sets
                    # has_written for the whole bank so the den matmuls can
                    # accumulate with start=False (the bank-wide clear of
                    # start=True would race the col-tiled pair).
                    bc = ps_c.tile([128, 512], f32, tag="bc", bufs=2, name=f"bc{j}_{hp}")
                    nc.tensor.matmul(
                        bc, zeros128, ats[0][:, 0:512],
                        start=True, stop=False, skip_group_check=True,
                    )
                    for hh in range(2):
                        for rc in range(2):
                            nc.tensor.matmul(
                                bc[hh * 64 : (hh + 1) * 64, :],
                                ones_blk,
                                ats[hh][:, rc * 512 : (rc + 1) * 512],
                                start=False,
                                stop=(rc == 1),
                                skip_group_check=True,
                            )
                    # reciprocal as exp(-ln(den)): ln and exp live in the
                    # SAME ACT table set ("natural_log_exp_and_others") so
                    # no table reloads between these and the softmax exps
                    # (Reciprocal needs its own set -> 1.3us reload each
                    # time the ACT stream alternates).
                    ld = p_bc.tile([128, 512], f32, tag="ld", name=f"ld{j}_{hp}")
                    nc.scalar.activation(
                        ld, bc, mybir.ActivationFunctionType.Ln
                    )
                    rec_sb = p_bc.tile([128, 512], cdt, tag="bcs", name=f"rec{j}_{hp}")
                    nc.scalar.activation(
                        rec_sb, ld, mybir.ActivationFunctionType.Exp, scale=-1.0
                    )
                    recs.append(rec_sb)
                    if debug and j == 0 and hp == 0:
                        dbg_sb = p_bc.tile([1, 4096], f32, tag="dbg", name="dbg_sb")
                        nc.scalar.copy(dbg_sb[0:1, 0:512], bc[0:1, :])
                        nc.scalar.copy(dbg_sb[0:1, 512:1024], bc[64:65, :])
                        nc.vector.tensor_copy(dbg_sb[0:1, 1024:1536], rec_sb[0:1, :])
                        nc.vector.tensor_copy(dbg_sb[0:1, 1536:2048], rec_sb[64:65, :])
                        nc.sync.dma_start(out=dbg_d, in_=dbg_sb)

                # pass 2: PV + normalize per pair
                for hp in range(4):
                    ats = at_j[hp * 2 : hp * 2 + 2]
                    # PV pair -> two [128, 512] PSUM tiles: head hh=0 at
                    # rows 0..63 of tile A, hh=1 at rows 64..127 of tile B
                    # (distinct col-groups AND banks -> concurrent on PE).
                    ops = [
                        ps_c.tile([128, 512], f32, tag="op", bufs=2, name=f"op{j}_{hp}_{hh}")
                        for hh in range(2)
                    ]
                    for hh in range(2):
                        h = hp * 2 + hh
                        for rc in range(2):
                            nc.tensor.matmul(
                                ops[hh][hh * 64 : hh * 64 + 64, :],
                                vext[:, h, rc, :],
                                ats[hh][:, rc * 512 : (rc + 1) * 512],
                                start=(rc == 0),
                                stop=(rc == 1),
                            )
                    for hh in range(2):
                        nc.vector.tensor_mul(
                            oT[hp][hh * 64 : (hh + 1) * 64, :],
                            ops[hh][hh * 64 : hh * 64 + 64, :],
                            recs[hp][hh * 64 : (hh + 1) * 64, :],
                        )

                # y tiles: [128, 512] per n-subchunk; fp32 + bias via DVE
                for s in range(4):
                    fp = ps_c.tile([128, 512], f32, tag="op", bufs=2, name=f"fp{j}_{s}")
                    for dm in range(4):
                        nc.tensor.matmul(
                            fp,
                            oT[dm][:, s * 128 : (s + 1) * 128],
                            w_sb["wo"][:, dm, :],
                            start=(dm == 0),
                            stop=(dm == 3),
                        )
                    fin = p_fin.tile([128, 512], f32, tag="fin", name=f"fin_{j}_{s}")
                    nc.vector.tensor_add(fin, fp, bias_bc[:, 0, :])
                    nc.sync.dma_start(
                        out=y_r[:, j * 4 + s : j * 4 + s + 1, :], in_=fin.unsqueeze(1)
                    )

    nc.compile()
    _built["nc"] = nc
    return nc


def prep_ef(E):
    """[NH, SEQ, R] -> [SEQ//128, 128, NH, R] bf16 (one contiguous block per
    128-row seq tile)."""
    np_c = ml_dtypes.bfloat16
    e = np.asarray(E).reshape(NH, SEQ // 128, 128, R)
    return np.ascontiguousarray(e.transpose(1, 2, 0, 3), dtype=np_c)


def _runner():
    """Build (once) a cached jitted 8-core executor for the Bass module."""
    if "run" in _built:
        return _built["run"]

    import jax
    import numpy as _np

    import concourse.mybir as mybir
    from concourse import bass2jax

    bass2jax.install_neuronx_cc_hook()
    nc = _build()

    part_name = nc.partition_id_tensor.name if nc.partition_id_tensor else None
    in_names, out_names, out_avals = [], [], []
    for alloc in nc.m.functions[0].allocations:
        if not isinstance(alloc, mybir.MemoryLocationSet):
            continue
        name = alloc.memorylocations[0].name
        if alloc.kind == "ExternalInput":
            if name != part_name:
                in_names.append(name)
        elif alloc.kind == "ExternalOutput":
            out_names.append(name)
            out_avals.append(
                jax.core.ShapedArray(
                    tuple(alloc.tensor_shape), mybir.dt.np(alloc.dtype)
                )
            )
    n_outs = len(out_avals)
    all_in_names = tuple(
        in_names + out_names + ([part_name] if part_name else [])
    )

    from jax.sharding import NamedSharding

    def _body(*args):
        operands = list(args)
        if part_name is not None:
            operands.append(bass2jax.partition_id_tensor())
        outs = bass2jax._bass_exec_p.bind(
            *operands,
            out_avals=tuple(out_avals),
            in_names=all_in_names,
            out_names=tuple(out_names),
            lowering_input_output_aliases=(),
            sim_require_finite=True,
            sim_require_nnan=True,
            nc=nc,
        )
        return tuple(outs)

    devices = jax.devices()[:NCORES]
    mesh = bass2jax.Mesh(_np.asarray(devices), ("core",))
    p_core = bass2jax.PartitionSpec("core")
    p_repl = bass2jax.PartitionSpec()
    # "x" is per-core; every other input is replicated across cores.
    # zero output buffers ride along as per-core params (hook requires params).
    in_specs = tuple(p_core if n == "x" else p_repl for n in in_names) + (
        p_core,
    ) * n_outs
    sharded = jax.jit(
        bass2jax.shard_map(
            _body,
            mesh=mesh,
            in_specs=in_specs,
            out_specs=(p_core,) * n_outs,
            check_rep=False,
        ),
        keep_unused=True,
    )
    sh_core = NamedSharding(mesh, p_core)
    sh_repl = NamedSharding(mesh, p_repl)
    dev_cache = {}

    zero_cache = {}

    def run(in_maps):
        args = []
        for name in in_names:
            if name == "x":
                xc = np.concatenate([np.asarray(m[name]) for m in in_maps], axis=0)
                args.append(jax.device_put(xc, sh_core))
            else:
                a = np.asarray(in_maps[0][name])
                key = (name, a.shape, str(a.dtype), hash(a.tobytes()))
                if key not in dev_cache:
                    dev_cache.clear() if len(dev_cache) > 64 else None
                    dev_cache[key] = jax.device_put(a, sh_repl)
                args.append(dev_cache[key])
        for i, a in enumerate(out_avals):
            if i not in zero_cache:
                zero_cache[i] = jax.device_put(
                    np.zeros((NCORES * a.shape[0], *a.shape[1:]), a.dtype), sh_core
                )
            args.append(zero_cache[i])
        out_arrs = sharded(*args)
        return [
            {
                name: np.asarray(out_arrs[i]).reshape(
                    NCORES, *out_avals[i].shape
                )[c]
                for i, name in enumerate(out_names)
            }
            for c in range(NCORES)
        ]

    _built["run"] = run
    return run


def make_in_maps(x, wq, wk, wv, E, F, w_out, b_out):
    """Full inputs -> list of per-core input dicts in kernel layouts."""
    np_c = ml_dtypes.bfloat16
    shared = {
        "wq": np.ascontiguousarray(wq, dtype=np_c),
        "wk": np.ascontiguousarray(wk, dtype=np_c),
        "wv": np.ascontiguousarray(wv, dtype=np_c),
        "E": prep_ef(E),
        "F": prep_ef(F),
        "w_out": np.ascontiguousarray(w_out, dtype=np_c),
        "b_out": np.ascontiguousarray(b_out, dtype=np.float32),
    }
    return [
        {
            "x": np.ascontiguousarray(np.asarray(x[i]).T, dtype=np_c),
            **shared,
        }
        for i in range(NCORES)
    ]


def kernel(x, wq, wk, wv, E, F, w_out, b_out):
    """Full inputs in, full output out. Shards batch across 8 cores."""
    run = _runner()
    in_maps = make_in_maps(x, wq, wk, wv, E, F, w_out, b_out)
    results = run(in_maps)
    return np.stack([results[i]["y"] for i in range(NCORES)], axis=0)


if __name__ == "__main__":
    xs = {
        "x": np.random.randn(BATCH, SEQ, DM).astype(np.float32),
        "wq": np.random.randn(DM, DM).astype(np.float32) * 0.05,
        "wk": np.random.randn(DM, DM).astype(np.float32) * 0.05,
        "wv": np.random.randn(DM, DM).astype(np.float32) * 0.05,
        "E": np.random.randn(NH, SEQ, R).astype(np.float32) * 0.03,
        "F": np.random.randn(NH, SEQ, R).astype(np.float32) * 0.03,
        "w_out": np.random.randn(DM, DM).astype(np.float32) * 0.05,
        "b_out": np.zeros(DM, np.float32),
    }
    y = kernel(**xs)
    print(y.shape, y.dtype)


# revision 31
# speedup vs baseline: 1.3420x; 1.3420x over previous
"""Linformer multi-head attention on 8 Trainium2 NeuronCores.

Sharding: data-parallel over batch (BATCH=8 -> 1 batch element per core).
Each core runs the full per-batch computation:
  q = x@wq, k = x@wk, v = x@wv            (per head h: 64-dim slices)
  k_proj[h] = E[h].T @ k[h]   [256, 64]   (contraction over seq)
  v_proj[h] = F[h].T @ v[h]   [256, 64]
  scores = q @ k_proj.T / 8   [4096, 256]
  attn = softmax(scores)  ;  out = attn @ v_proj
  y = concat_heads(out) @ w_out + b_out

v2 design notes (vs v1 at 506us):
  - x is transposed on HOST -> xT [512, 4096]; no on-chip transposes.
  - E/F are relayouted on HOST to [32 tiles, 128, 8 heads, 256] so each
    (j, s) DMA is one fully-contiguous 512KB block.
  - k_projT/v_projT accumulate in 4 persistent PSUM banks across all 32
    seq-tiles (no DVE partial adds).  A zero-matmul initializes each bank
    (has_written set everywhere) so every real matmul uses start=False --
    avoids the bank-wide has_written clear racing between interleaved
    accumulation regions.
  - M=64 kp/vp matmuls and K=64 score matmuls run as tile_position pairs
    (col/row-group concurrency, ~2x).
  - softmax denominator comes free from the PV matmul via an appended
    ones-column (row 64); per head-PAIR the two PV outputs land in one
    [128, 1024] PSUM tile so one reciprocal_approx_fast [1, 1024] handles
    both heads (v1 used full-precision reciprocal: 3.3us/op, 213us total).
  - reciprocal -> broadcast via rank-1 PE matmuls in float32r (full rate
    at N=512; plain f32 matmul is 4x slower).
  - evacuation work split between ScalarE (qt, bc) and VectorE (kv, oT
    muls, fin bias-adds); exp on ScalarE in [128, 1024] ops.

Compute dtype is bf16 (inputs cast on host) with fp32 PSUM accumulation.
"""

import os

import numpy as np
import ml_dtypes

BATCH, SEQ, DM = 8, 4096, 512
NH, DH, R = 8, 64, 256
NCORES = 8
NT = SEQ // 512  # 8 big n-tiles of 512 rows

_built = {}


def _build():
    """Build the Bass module (once per process)."""
    if "nc" in _built:
        return _built["nc"]

    from contextlib import ExitStack

    import concourse.bass as bass
    import concourse.bacc as bacc
    import concourse.mybir as mybir
    import concourse.tile as tile
    from concourse.masks import make_identity

    f32 = mybir.dt.float32
    f32r = mybir.dt.float32r
    cdt = mybir.dt.bfloat16

    nc = bacc.Bacc("TRN2", target_bir_lowering=False, debug=False)

    # xT: host-transposed [DM, SEQ]
    x_d = nc.dram_tensor("x", [DM, SEQ], cdt, kind="ExternalInput").ap()
    wq_d = nc.dram_tensor("wq", [DM, DM], cdt, kind="ExternalInput").ap()
    wk_d = nc.dram_tensor("wk", [DM, DM], cdt, kind="ExternalInput").ap()
    wv_d = nc.dram_tensor("wv", [DM, DM], cdt, kind="ExternalInput").ap()
    # E/F host layout: [ti, p, h, r] with ti = j*4+s, seq = ti*128+p
    e_d = nc.dram_tensor("E", [SEQ // 128, 128, NH, R], cdt, kind="ExternalInput").ap()
    f_d = nc.dram_tensor("F", [SEQ // 128, 128, NH, R], cdt, kind="ExternalInput").ap()
    wo_d = nc.dram_tensor("w_out", [DM, DM], cdt, kind="ExternalInput").ap()
    b_d = nc.dram_tensor("b_out", [DM], f32, kind="ExternalInput").ap()
    y_d = nc.dram_tensor("y", [SEQ, DM], f32, kind="ExternalOutput").ap()
    debug = os.environ.get("LINF_DEBUG", "0") == "1"
    if debug:
        dbg_d = nc.dram_tensor("dbg", [1, 4096], f32, kind="ExternalOutput").ap()

    with tile.TileContext(nc) as tc, ExitStack() as ctx:
        singles = ctx.enter_context(tc.tile_pool(name="singles", bufs=1))

        ident = singles.tile([128, 128], cdt)
        make_identity(nc, ident)
        ones_blk = singles.tile([128, 64], cdt)
        nc.vector.memset(ones_blk, 1.0)
        zeros128 = singles.tile([128, 128], cdt)
        nc.vector.memset(zeros128, 0.0)

        def act_recip(out, in_):
            """ACT Reciprocal LUT (bass blocks it for accuracy; softmax
            denominators only need ~1e-2 so the LUT is fine here)."""
            eng = nc.scalar
            ins = [eng.lower_ap(in_)]
            for val in (0.0, 1.0, 0.0):  # bias, scale, alpha
                ins.append(mybir.ImmediateValue(dtype=f32, value=val))
            return eng.add_instruction(
                mybir.InstActivation(
                    name=nc.get_next_instruction_name(),
                    func=mybir.ActivationFunctionType.Reciprocal,
                    ins=ins,
                    outs=[eng.lower_ap(out)],
                )
            )
        # bias replicated [128, 2, 512] for the [128, 1024] fin bias-add
        bias_bc = singles.tile([128, 2, DM], f32)
        b_bc_ap = bass.AP(
            tensor=b_d.tensor,
            offset=b_d.offset,
            ap=[[0, 128], [0, 2]] + list(b_d.ap),
        )
        nc.sync.dma_start(out=bias_bc, in_=b_bc_ap)

        # weights as [128, dk, 512]: chunk dk holds rows dk*128..+128
        w_sb = {}
        for name, d in (("wq", wq_d), ("wk", wk_d), ("wv", wv_d), ("wo", wo_d)):
            t = singles.tile([128, 4, DM], cdt, name=f"w_{name}")
            nc.sync.dma_start(out=t, in_=d.rearrange("(dk p) m -> p dk m", p=128))
            w_sb[name] = t

        # QT global [512, 4096] as 4 tiles [128, 4096]; tile t = heads 2t,2t+1
        qt_g = [singles.tile([128, SEQ], cdt, tag=f"qt{t}", name=f"qt{t}") for t in range(4)]
        # per-head low-rank projections, transposed [64, 256], packed 4/tile:
        # head h -> tile t=h//4, partition half ph=h%2, col half ch=(h//2)%2
        kpT_sb = [singles.tile([128, 2 * R], cdt, tag=f"kp{t}", name=f"kpT{t}") for t in range(2)]
        vpT_sb = [singles.tile([128, 2 * R], cdt, tag=f"vp{t}", name=f"vpT{t}") for t in range(2)]

        def hslice(sb, h):
            """[64, 256] slice of packed kpT/vpT for head h."""
            t, ph, ch = h // 4, h % 2, (h // 2) % 2
            return sb[t][ph * 64 : (ph + 1) * 64, ch * R : (ch + 1) * R]

        # v_proj natural chunks: [128, 2, 64] per head
        vext = singles.tile([128, NH, 2, 64], cdt)

        # ---------------- Phase AB: QT, k_projT, v_projT ----------------
        with (
            tc.tile_pool(name="p_x", bufs=3) as p_x,
            tc.tile_pool(name="p_ef", bufs=3) as p_ef,
            tc.tile_pool(name="p_kv", bufs=6) as p_kv,
            tc.tile_pool(name="ps_acc", bufs=1, space="PSUM") as ps_acc,
            tc.tile_pool(name="ps_mm", bufs=4, space="PSUM") as ps_mm,
        ):
            # persistent PSUM accumulators: 4 banks, live all of phase AB
            kpT_ps = [
                ps_acc.tile([128, 2 * R], f32, tag=f"kpp{t}", name=f"kpT_ps{t}")
                for t in range(2)
            ]
            vpT_ps = [
                ps_acc.tile([128, 2 * R], f32, tag=f"vpp{t}", name=f"vpT_ps{t}")
                for t in range(2)
            ]
            # init: one full-bank zero-matmul (lhsT=0 so rhs content is
            # irrelevant) sets has_written on every element so all real
            # accumulation matmuls can use start=False (see header).
            for t in range(2):
                for acc in (kpT_ps[t], vpT_ps[t]):
                    nc.tensor.matmul(
                        acc,
                        zeros128,
                        w_sb["wq"][:, 0, :],
                        start=True,
                        stop=False,
                        skip_group_check=True,
                    )

            x_r = x_d.rearrange("(dk p) n -> p dk n", p=128)
            e_r = e_d.rearrange("t p h r -> p t h r")
            f_r = f_d.rearrange("t p h r -> p t h r")
            for j in range(NT):  # 8 n-tiles of 512 rows
                xt = p_x.tile([128, 4, 512], cdt, tag="xt", name=f"xt_{j}")
                nc.sync.dma_start(out=xt, in_=x_r[:, :, j * 512 : (j + 1) * 512])

                e_t = p_ef.tile([128, 4, NH, R], cdt, tag="ef_e", name=f"e_{j}")
                f_t = p_ef.tile([128, 4, NH, R], cdt, tag="ef_f", name=f"f_{j}")
                nc.sync.dma_start(out=e_t, in_=e_r[:, j * 4 : (j + 1) * 4, :, :])
                nc.sync.dma_start(out=f_t, in_=f_r[:, j * 4 : (j + 1) * 4, :, :])
                e_ts = [e_t[:, s, :, :] for s in range(4)]
                f_ts = [f_t[:, s, :, :] for s in range(4)]

                # QT_j[dq] [128, 512] = sum_dk wq[dk, dq-chunk].T-form @ xT[dk]
                for dq in range(4):
                    pq = ps_mm.tile([128, 512], f32, tag="pmm", name=f"pq_{j}_{dq}")
                    for dk in range(4):
                        nc.tensor.matmul(
                            pq,
                            w_sb["wq"][:, dk, dq * 128 : (dq + 1) * 128],
                            xt[:, dk, :],
                            start=(dk == 0),
                            stop=(dk == 3),
                        )
                    nc.scalar.copy(qt_g[dq][:, j * 512 : (j + 1) * 512], pq)

                # K/V per 128-row subtile s, then accumulate projections
                for s in range(4):
                    for wname, ef in (("wk", e_ts[s]), ("wv", f_ts[s])):
                        pk = ps_mm.tile([128, 512], f32, tag="pmm", name=f"pk_{j}_{s}")
                        for dk in range(4):
                            nc.tensor.matmul(
                                pk,
                                xt[:, dk, s * 128 : (s + 1) * 128],
                                w_sb[wname][:, dk, :],
                                start=(dk == 0),
                                stop=(dk == 3),
                            )
                        kv_sb = p_kv.tile([128, 512], cdt, tag="kv", name=f"kv_{j}_{s}")
                        nc.vector.tensor_copy(kv_sb, pk)

                        acc = kpT_ps if wname == "wk" else vpT_ps
                        last = (j == NT - 1) and (s == 3)
                        # col-tiled pairs: heads (2i, 2i+1) -> partition
                        # halves 0/64 of the same bank, concurrent on PE.
                        for h in range(NH):
                            t, ph, ch = h // 4, h % 2, (h // 2) % 2
                            nc.tensor.matmul(
                                acc[t][
                                    ph * 64 : (ph + 1) * 64,
                                    ch * R : (ch + 1) * R,
                                ],
                                kv_sb[:, h * 64 : (h + 1) * 64],
                                ef[:, h, :],
                                start=False,
                                stop=last,
                                skip_group_check=True,
                            )

            for t in range(2):
                nc.scalar.copy(kpT_sb[t], kpT_ps[t])
                nc.scalar.copy(vpT_sb[t], vpT_ps[t])

        # ---------------- Phase C: attention + output dense ----------------
        y_r = y_d.rearrange("(t p) m -> p t m", p=128)  # t = j*4+s
        with (
            tc.tile_pool(name="p_at", bufs=10) as p_at,
            tc.tile_pool(name="p_bc", bufs=6) as p_bc,
            tc.tile_pool(name="p_ot", bufs=8) as p_ot,
            tc.tile_pool(name="p_fin", bufs=4) as p_fin,
            tc.tile_pool(name="ps_c", bufs=2, space="PSUM") as ps_c,
        ):
            # build vext: transpose v_projT[h] chunks to natural
            for h in range(NH):
                pv = ps_c.tile([128, 512], cdt, tag="op", bufs=2, name="pv")
                for rc in range(2):
                    nc.tensor.transpose(
                        pv[:, rc * 64 : (rc + 1) * 64],
                        hslice(vpT_sb, h)[:, rc * 128 : (rc + 1) * 128],
                        ident[(h % 2) * 64 : (h % 2) * 64 + 64, (h % 2) * 64 : (h % 2) * 64 + 64],
                    )
                for rc in range(2):
                    nc.vector.tensor_copy(
                        vext[:, h, rc, :], pv[:, rc * 64 : (rc + 1) * 64]
                    )

            for j in range(NT):
                oT = [p_ot.tile([128, 512], cdt, tag="ot", name=f"oT{j}_{t}") for t in range(4)]
                # all 8 heads' scores+exp first so the ACT stream runs the 8
                # Exp ops back-to-back, then the 4 Reciprocal ops: 2 ACT
                # table switches per j instead of 8 (each reload is 1.3us).
                at_j = []
                for h in range(NH):
                    ph = h % 2
                    qrow = qt_g[h // 2][
                        ph * 64 : ph * 64 + 64, j * 512 : (j + 1) * 512
                    ]
                    # scoresT [256, 512] as one [128, 1024] tile
                    # (rc chunks in col halves); K=64 row-tiled pair
                    # with the other head of hp runs concurrently.
                    sc = ps_c.tile([128, 1024], f32, tag="sc", name=f"sc{j}_{h}")
                    for rc in range(2):
                        nc.tensor.matmul(
                            sc[:, rc * 512 : (rc + 1) * 512],
                            hslice(kpT_sb, h)[:, rc * 128 : (rc + 1) * 128],
                            qrow,
                            start=True,
                            stop=True,
                        )
                    a = p_at.tile([128, 1024], cdt, tag="at", name=f"at{j}_{h}")
                    nc.scalar.activation(
                        a, sc, mybir.ActivationFunctionType.Exp, scale=0.125
                    )
                    at_j.append(a)

                # pass 1: denominators for all 4 pairs (needs only at_j) so
                # the ACT reciprocals run right after the exps -- ACT
                # stream is [exp x8, recip x2] per j (2 table switches).
                # Two pairs share one [128, 1024] den tile (one bank per
                # pair) -> one Reciprocal op covers both.
                recs = []
                for g in range(2):  # pair groups (hp = 2g, 2g+1)
                    bc = ps_c.tile([128, 1024], f32, tag="bc", bufs=1, name=f"bc{j}_{g}")
                    for q in range(2):
                        hp = g * 2 + q
                        ats = at_j[hp * 2 : hp * 2 + 2]
                        bank = bc[:, q * 512 : (q + 1) * 512]
                        # den broadcast via all-ones stationary matmuls over
                        # attnT: rows 0..63 = den_h0, rows 64..127 = den_h1
                        # (col-tiled concurrent).  A zero-matmul sets
                        # has_written for the whole bank so the den matmuls
                        # can accumulate with start=False (the bank-wide
                        # clear of start=True would race the col-tiled
                        # pair).
                        nc.tensor.matmul(
                            bank, zeros128, ats[0][:, 0:512],
                            start=True, stop=False, skip_group_check=True,
                        )
                        for hh in range(2):
                            for rc in range(2):
                                nc.tensor.matmul(
                                    bank[hh * 64 : (hh + 1) * 64, :],
                                    ones_blk,
                                    ats[hh][:, rc * 512 : (rc + 1) * 512],
                                    start=False,
                                    stop=(rc == 1),
                                    skip_group_check=True,
                                )
                    # evacuation doubles as the reciprocal: rec = 1/den
                    rec_sb = p_bc.tile([128, 1024], cdt, tag="bcs", name=f"rec{j}_{g}")
                    act_recip(rec_sb, bc)
                    recs.append(rec_sb)
                    if debug and j == 0 and g == 0:
                        dbg_sb = p_bc.tile([1, 4096], f32, tag="dbg", name="dbg_sb")
                        nc.scalar.copy(dbg_sb[0:1, 0:512], bc[0:1, 0:512])
                        nc.scalar.copy(dbg_sb[0:1, 512:1024], bc[64:65, 0:512])
                        nc.vector.tensor_copy(dbg_sb[0:1, 1024:1536], rec_sb[0:1, 0:512])
                        nc.vector.tensor_copy(dbg_sb[0:1, 1536:2048], rec_sb[64:65, 0:512])
                        nc.sync.dma_start(out=dbg_d, in_=dbg_sb)

                # pass 2: PV + normalize per pair
                for hp in range(4):
                    ats = at_j[hp * 2 : hp * 2 + 2]
                    # PV pair -> two [128, 512] PSUM tiles: head hh=0 at
                    # rows 0..63 of tile A, hh=1 at rows 64..127 of tile B
                    # (distinct col-groups AND banks -> concurrent on PE).
                    ops = [
                        ps_c.tile([128, 512], f32, tag="op", bufs=2, name=f"op{j}_{hp}_{hh}")
                        for hh in range(2)
                    ]
                    for hh in range(2):
                        h = hp * 2 + hh
                        for rc in range(2):
                            nc.tensor.matmul(
                                ops[hh][hh * 64 : hh * 64 + 64, :],
                                vext[:, h, rc, :],
                                ats[hh][:, rc * 512 : (rc + 1) * 512],
                                start=(rc == 0),
                                stop=(rc == 1),
                            )
                    for hh in range(2):
                        nc.vector.tensor_mul(
                            oT[hp][hh * 64 : (hh + 1) * 64, :],
                            ops[hh][hh * 64 : hh * 64 + 64, :],
                            recs[hp // 2][
                                hh * 64 : (hh + 1) * 64,
                                (hp % 2) * 512 : (hp % 2 + 1) * 512,
                            ],
                        )

                # y tiles: [128, 512] per n-subchunk; fp32 + bias via DVE
                for s in range(4):
                    fp = ps_c.tile([128, 512], f32, tag="op", bufs=2, name=f"fp{j}_{s}")
                    for dm in range(4):
                        nc.tensor.matmul(
                            fp,
                            oT[dm][:, s * 128 : (s + 1) * 128],
                            w_sb["wo"][:, dm, :],
                            start=(dm == 0),
                            stop=(dm == 3),
                        )
                    fin = p_fin.tile([128, 512], f32, tag="fin", name=f"fin_{j}_{s}")
                    nc.vector.tensor_add(fin, fp, bias_bc[:, 0, :])
                    nc.sync.dma_start(
                        out=y_r[:, j * 4 + s : j * 4 + s + 1, :], in_=fin.unsqueeze(1)
                    )

    nc.compile()
    _built["nc"] = nc
    return nc


def prep_ef(E):
    """[NH, SEQ, R] -> [SEQ//128, 128, NH, R] bf16 (one contiguous block per
    128-row seq tile)."""
    np_c = ml_dtypes.bfloat16
    e = np.asarray(E).reshape(NH, SEQ // 128, 128, R)
    return np.ascontiguousarray(e.transpose(1, 2, 0, 3), dtype=np_c)


def _runner():
    """Build (once) a cached jitted 8-core executor for the Bass module."""
    if "run" in _built:
        return _built["run"]

    import jax
    import numpy as _np

    import concourse.mybir as mybir
    from concourse import bass2jax

    bass2jax.install_neuronx_cc_hook()
    nc = _build()

    part_name = nc.partition_id_tensor.name if nc.partition_id_tensor else None
    in_names, out_names, out_avals = [], [], []
    for alloc in nc.m.functions[0].allocations:
        if not isinstance(alloc, mybir.MemoryLocationSet):
            continue
        name = alloc.memorylocations[0].name
        if alloc.kind == "ExternalInput":
            if name != part_name:
                in_names.append(name)
        elif alloc.kind == "ExternalOutput":
            out_names.append(name)
            out_avals.append(
                jax.core.ShapedArray(
                    tuple(alloc.tensor_shape), mybir.dt.np(alloc.dtype)
                )
            )
    n_outs = len(out_avals)
    all_in_names = tuple(
        in_names + out_names + ([part_name] if part_name else [])
    )

    from jax.sharding import NamedSharding

    def _body(*args):
        operands = list(args)
        if part_name is not None:
            operands.append(bass2jax.partition_id_tensor())
        outs = bass2jax._bass_exec_p.bind(
            *operands,
            out_avals=tuple(out_avals),
            in_names=all_in_names,
            out_names=tuple(out_names),
            lowering_input_output_aliases=(),
            sim_require_finite=True,
            sim_require_nnan=True,
            nc=nc,
        )
        return tuple(outs)

    devices = jax.devices()[:NCORES]
    mesh = bass2jax.Mesh(_np.asarray(devices), ("core",))
    p_core = bass2jax.PartitionSpec("core")
    p_repl = bass2jax.PartitionSpec()
    # "x" is per-core; every other input is replicated across cores.
    # zero output buffers ride along as per-core params (hook requires params).
    in_specs = tuple(p_core if n == "x" else p_repl for n in in_names) + (
        p_core,
    ) * n_outs
    sharded = jax.jit(
        bass2jax.shard_map(
            _body,
            mesh=mesh,
            in_specs=in_specs,
            out_specs=(p_core,) * n_outs,
            check_rep=False,
        ),
        keep_unused=True,
    )
    sh_core = NamedSharding(mesh, p_core)
    sh_repl = NamedSharding(mesh, p_repl)
    dev_cache = {}

    zero_cache = {}

    def run(in_maps):
        args = []
        for name in in_names:
            if name == "x":
                xc = np.concatenate([np.asarray(m[name]) for m in in_maps], axis=0)
                args.append(jax.device_put(xc, sh_core))
            else:
                a = np.asarray(in_maps[0][name])
                key = (name, a.shape, str(a.dtype), hash(a.tobytes()))
                if key not in dev_cache:
                    dev_cache.clear() if len(dev_cache) > 64 else None
                    dev_cache[key] = jax.device_put(a, sh_repl)
                args.append(dev_cache[key])
        for i, a in enumerate(out_avals):
            if i not in zero_cache:
                zero_cache[i] = jax.device_put(
                    np.zeros((NCORES * a.shape[0], *a.shape[1:]), a.dtype), sh_core
                )
            args.append(zero_cache[i])
        out_arrs = sharded(*args)
        return [
            {
                name: np.asarray(out_arrs[i]).reshape(
                    NCORES, *out_avals[i].shape
                )[c]
                for i, name in enumerate(out_names)
            }
            for c in range(NCORES)
        ]

    _built["run"] = run
    return run


def make_in_maps(x, wq, wk, wv, E, F, w_out, b_out):
    """Full inputs -> list of per-core input dicts in kernel layouts."""
    np_c = ml_dtypes.bfloat16
    shared = {
        "wq": np.ascontiguousarray(wq, dtype=np_c),
        "wk": np.ascontiguousarray(wk, dtype=np_c),
        "wv": np.ascontiguousarray(wv, dtype=np_c),
        "E": prep_ef(E),
        "F": prep_ef(F),
        "w_out": np.ascontiguousarray(w_out, dtype=np_c),
        "b_out": np.ascontiguousarray(b_out, dtype=np.float32),
    }
    return [
        {
            "x": np.ascontiguousarray(np.asarray(x[i]).T, dtype=np_c),
            **shared,
        }
        for i in range(NCORES)
    ]


def kernel(x, wq, wk, wv, E, F, w_out, b_out):
    """Full inputs in, full output out. Shards batch across 8 cores."""
    run = _runner()
    in_maps = make_in_maps(x, wq, wk, wv, E, F, w_out, b_out)
    results = run(in_maps)
    return np.stack([results[i]["y"] for i in range(NCORES)], axis=0)


if __name__ == "__main__":
    xs = {
        "x": np.random.randn(BATCH, SEQ, DM).astype(np.float32),
        "wq": np.random.randn(DM, DM).astype(np.float32) * 0.05,
        "wk": np.random.randn(DM, DM).astype(np.float32) * 0.05,
        "wv": np.random.randn(DM, DM).astype(np.float32) * 0.05,
        "E": np.random.randn(NH, SEQ, R).astype(np.float32) * 0.03,
        "F": np.random.randn(NH, SEQ, R).astype(np.float32) * 0.03,
        "w_out": np.random.randn(DM, DM).astype(np.float32) * 0.05,
        "b_out": np.zeros(DM, np.float32),
    }
    y = kernel(**xs)
    print(y.shape, y.dtype)


# revision 35
# speedup vs baseline: 1.4266x; 1.0630x over previous
"""Linformer multi-head attention on 8 Trainium2 NeuronCores.

Sharding: data-parallel over batch (BATCH=8 -> 1 batch element per core).
Each core runs the full per-batch computation:
  q = x@wq, k = x@wk, v = x@wv            (per head h: 64-dim slices)
  k_proj[h] = E[h].T @ k[h]   [256, 64]   (contraction over seq)
  v_proj[h] = F[h].T @ v[h]   [256, 64]
  scores = q @ k_proj.T / 8   [4096, 256]
  attn = softmax(scores)  ;  out = attn @ v_proj
  y = concat_heads(out) @ w_out + b_out

v2 design notes (vs v1 at 506us):
  - x is transposed on HOST -> xT [512, 4096]; no on-chip transposes.
  - E/F are relayouted on HOST to [32 tiles, 128, 8 heads, 256] so each
    (j, s) DMA is one fully-contiguous 512KB block.
  - k_projT/v_projT accumulate in 4 persistent PSUM banks across all 32
    seq-tiles (no DVE partial adds).  A zero-matmul initializes each bank
    (has_written set everywhere) so every real matmul uses start=False --
    avoids the bank-wide has_written clear racing between interleaved
    accumulation regions.
  - M=64 kp/vp matmuls and K=64 score matmuls run as tile_position pairs
    (col/row-group concurrency, ~2x).
  - softmax denominator comes free from the PV matmul via an appended
    ones-column (row 64); per head-PAIR the two PV outputs land in one
    [128, 1024] PSUM tile so one reciprocal_approx_fast [1, 1024] handles
    both heads (v1 used full-precision reciprocal: 3.3us/op, 213us total).
  - reciprocal -> broadcast via rank-1 PE matmuls in float32r (full rate
    at N=512; plain f32 matmul is 4x slower).
  - evacuation work split between ScalarE (qt, bc) and VectorE (kv, oT
    muls, fin bias-adds); exp on ScalarE in [128, 1024] ops.

Compute dtype is bf16 (inputs cast on host) with fp32 PSUM accumulation.
"""

import os

import numpy as np
import ml_dtypes

BATCH, SEQ, DM = 8, 4096, 512
NH, DH, R = 8, 64, 256
NCORES = 8
NT = SEQ // 512  # 8 big n-tiles of 512 rows

_built = {}


def _build():
    """Build the Bass module (once per process)."""
    if "nc" in _built:
        return _built["nc"]

    from contextlib import ExitStack

    import concourse.bass as bass
    import concourse.bacc as bacc
    import concourse.mybir as mybir
    import concourse.tile as tile
    from concourse.masks import make_identity

    f32 = mybir.dt.float32
    f32r = mybir.dt.float32r
    cdt = mybir.dt.bfloat16

    nc = bacc.Bacc("TRN2", target_bir_lowering=False, debug=False)

    # xT: host-transposed [DM, SEQ]
    x_d = nc.dram_tensor("x", [DM, SEQ], cdt, kind="ExternalInput").ap()
    wq_d = nc.dram_tensor("wq", [DM, DM], cdt, kind="ExternalInput").ap()
    wk_d = nc.dram_tensor("wk", [DM, DM], cdt, kind="ExternalInput").ap()
    wv_d = nc.dram_tensor("wv", [DM, DM], cdt, kind="ExternalInput").ap()
    # E/F host layout: [ti, p, h, r] with ti = j*4+s, seq = ti*128+p
    e_d = nc.dram_tensor("E", [SEQ // 128, 128, NH, R], cdt, kind="ExternalInput").ap()
    f_d = nc.dram_tensor("F", [SEQ // 128, 128, NH, R], cdt, kind="ExternalInput").ap()
    wo_d = nc.dram_tensor("w_out", [DM, DM], cdt, kind="ExternalInput").ap()
    b_d = nc.dram_tensor("b_out", [DM], f32, kind="ExternalInput").ap()
    y_d = nc.dram_tensor("y", [SEQ, DM], f32, kind="ExternalOutput").ap()
    debug = os.environ.get("LINF_DEBUG", "0") == "1"
    if debug:
        dbg_d = nc.dram_tensor("dbg", [1, 4096], f32, kind="ExternalOutput").ap()

    with tile.TileContext(nc) as tc, ExitStack() as ctx:
        singles = ctx.enter_context(tc.tile_pool(name="singles", bufs=1))

        ident = singles.tile([128, 128], cdt)
        make_identity(nc, ident)
        ones_blk = singles.tile([128, 64], cdt)
        nc.vector.memset(ones_blk, 1.0)
        zeros128 = singles.tile([128, 128], cdt)
        nc.vector.memset(zeros128, 0.0)

        def act_recip(out, in_):
            """ACT Reciprocal LUT (bass blocks it for accuracy; softmax
            denominators only need ~1e-2 so the LUT is fine here)."""
            eng = nc.scalar
            ins = [eng.lower_ap(in_)]
            for val in (0.0, 1.0, 0.0):  # bias, scale, alpha
                ins.append(mybir.ImmediateValue(dtype=f32, value=val))
            return eng.add_instruction(
                mybir.InstActivation(
                    name=nc.get_next_instruction_name(),
                    func=mybir.ActivationFunctionType.Reciprocal,
                    ins=ins,
                    outs=[eng.lower_ap(out)],
                )
            )
        # bias replicated [128, 2, 512] for the [128, 1024] fin bias-add
        bias_bc = singles.tile([128, 2, DM], f32)
        b_bc_ap = bass.AP(
            tensor=b_d.tensor,
            offset=b_d.offset,
            ap=[[0, 128], [0, 2]] + list(b_d.ap),
        )
        nc.sync.dma_start(out=bias_bc, in_=b_bc_ap)

        # weights as [128, dk, 512]: chunk dk holds rows dk*128..+128
        w_sb = {}
        for name, d in (("wq", wq_d), ("wk", wk_d), ("wv", wv_d), ("wo", wo_d)):
            t = singles.tile([128, 4, DM], cdt, name=f"w_{name}")
            nc.sync.dma_start(out=t, in_=d.rearrange("(dk p) m -> p dk m", p=128))
            w_sb[name] = t

        # QT global [512, 4096] as 4 tiles [128, 4096]; tile t = heads 2t,2t+1
        qt_g = [singles.tile([128, SEQ], cdt, tag=f"qt{t}", name=f"qt{t}") for t in range(4)]
        # per-head low-rank projections, transposed [64, 256], packed 4/tile:
        # head h -> tile t=h//4, partition half ph=h%2, col half ch=(h//2)%2
        kpT_sb = [singles.tile([128, 2 * R], cdt, tag=f"kp{t}", name=f"kpT{t}") for t in range(2)]
        vpT_sb = [singles.tile([128, 2 * R], cdt, tag=f"vp{t}", name=f"vpT{t}") for t in range(2)]

        def hslice(sb, h):
            """[64, 256] slice of packed kpT/vpT for head h."""
            t, ph, ch = h // 4, h % 2, (h // 2) % 2
            return sb[t][ph * 64 : (ph + 1) * 64, ch * R : (ch + 1) * R]

        # v_proj natural chunks: [128, 2, 64] per head
        vext = singles.tile([128, NH, 2, 64], cdt)

        # ---------------- Phase AB: QT, k_projT, v_projT ----------------
        with (
            tc.tile_pool(name="p_x", bufs=3) as p_x,
            tc.tile_pool(name="p_ef", bufs=3) as p_ef,
            tc.tile_pool(name="p_kv", bufs=6) as p_kv,
            tc.tile_pool(name="ps_acc", bufs=1, space="PSUM") as ps_acc,
            tc.tile_pool(name="ps_mm", bufs=4, space="PSUM") as ps_mm,
        ):
            # persistent PSUM accumulators: 4 banks, live all of phase AB
            kpT_ps = [
                ps_acc.tile([128, 2 * R], f32, tag=f"kpp{t}", name=f"kpT_ps{t}")
                for t in range(2)
            ]
            vpT_ps = [
                ps_acc.tile([128, 2 * R], f32, tag=f"vpp{t}", name=f"vpT_ps{t}")
                for t in range(2)
            ]
            # init: one full-bank zero-matmul (lhsT=0 so rhs content is
            # irrelevant) sets has_written on every element so all real
            # accumulation matmuls can use start=False (see header).
            for t in range(2):
                for acc in (kpT_ps[t], vpT_ps[t]):
                    nc.tensor.matmul(
                        acc,
                        zeros128,
                        w_sb["wq"][:, 0, :],
                        start=True,
                        stop=False,
                        skip_group_check=True,
                    )

            x_r = x_d.rearrange("(dk p) n -> p dk n", p=128)
            e_r = e_d.rearrange("t p h r -> p t h r")
            f_r = f_d.rearrange("t p h r -> p t h r")
            for j in range(NT):  # 8 n-tiles of 512 rows
                xt = p_x.tile([128, 4, 512], cdt, tag="xt", name=f"xt_{j}")
                nc.sync.dma_start(out=xt, in_=x_r[:, :, j * 512 : (j + 1) * 512])

                e_t = p_ef.tile([128, 4, NH, R], cdt, tag="ef_e", name=f"e_{j}")
                f_t = p_ef.tile([128, 4, NH, R], cdt, tag="ef_f", name=f"f_{j}")
                nc.sync.dma_start(out=e_t, in_=e_r[:, j * 4 : (j + 1) * 4, :, :])
                nc.sync.dma_start(out=f_t, in_=f_r[:, j * 4 : (j + 1) * 4, :, :])
                e_ts = [e_t[:, s, :, :] for s in range(4)]
                f_ts = [f_t[:, s, :, :] for s in range(4)]

                # QT_j[dq] [128, 512] = sum_dk wq[dk, dq-chunk].T-form @ xT[dk]
                for dq in range(4):
                    pq = ps_mm.tile([128, 512], f32, tag="pmm", name=f"pq_{j}_{dq}")
                    for dk in range(4):
                        nc.tensor.matmul(
                            pq,
                            w_sb["wq"][:, dk, dq * 128 : (dq + 1) * 128],
                            xt[:, dk, :],
                            start=(dk == 0),
                            stop=(dk == 3),
                        )
                    nc.scalar.copy(qt_g[dq][:, j * 512 : (j + 1) * 512], pq)

                # K/V per 128-row subtile s, then accumulate projections
                for s in range(4):
                    for wname, ef in (("wk", e_ts[s]), ("wv", f_ts[s])):
                        pk = ps_mm.tile([128, 512], f32, tag="pmm", name=f"pk_{j}_{s}")
                        for dk in range(4):
                            nc.tensor.matmul(
                                pk,
                                xt[:, dk, s * 128 : (s + 1) * 128],
                                w_sb[wname][:, dk, :],
                                start=(dk == 0),
                                stop=(dk == 3),
                            )
                        kv_sb = p_kv.tile([128, 512], cdt, tag="kv", name=f"kv_{j}_{s}")
                        nc.vector.tensor_copy(kv_sb, pk)

                        acc = kpT_ps if wname == "wk" else vpT_ps
                        last = (j == NT - 1) and (s == 3)
                        # col-tiled pairs: heads (2i, 2i+1) -> partition
                        # halves 0/64 of the same bank, concurrent on PE.
                        for h in range(NH):
                            t, ph, ch = h // 4, h % 2, (h // 2) % 2
                            nc.tensor.matmul(
                                acc[t][
                                    ph * 64 : (ph + 1) * 64,
                                    ch * R : (ch + 1) * R,
                                ],
                                kv_sb[:, h * 64 : (h + 1) * 64],
                                ef[:, h, :],
                                start=False,
                                stop=last,
                                skip_group_check=True,
                            )

            for t in range(2):
                nc.scalar.copy(kpT_sb[t], kpT_ps[t])
                nc.scalar.copy(vpT_sb[t], vpT_ps[t])

        # ---------------- Phase C: attention + output dense ----------------
        y_r = y_d.rearrange("(t p) m -> p t m", p=128)  # t = j*4+s
        with (
            tc.tile_pool(name="p_at", bufs=18) as p_at,
            tc.tile_pool(name="p_bc", bufs=6) as p_bc,
            tc.tile_pool(name="p_ot", bufs=8) as p_ot,
            tc.tile_pool(name="p_fin", bufs=4) as p_fin,
            tc.tile_pool(name="ps_c", bufs=2, space="PSUM") as ps_c,
        ):
            # build vext: transpose v_projT[h] chunks to natural
            for h in range(NH):
                pv = ps_c.tile([128, 512], cdt, tag="op", bufs=2, name="pv")
                for rc in range(2):
                    nc.tensor.transpose(
                        pv[:, rc * 64 : (rc + 1) * 64],
                        hslice(vpT_sb, h)[:, rc * 128 : (rc + 1) * 128],
                        ident[(h % 2) * 64 : (h % 2) * 64 + 64, (h % 2) * 64 : (h % 2) * 64 + 64],
                    )
                for rc in range(2):
                    nc.vector.tensor_copy(
                        vext[:, h, rc, :], pv[:, rc * 64 : (rc + 1) * 64]
                    )

            def emit_scores_exp(j):
                """Scores + exp for all 8 heads of tile j.  Batched so the
                ACT stream runs the 8 Exp ops back-to-back, then the 2
                Reciprocal ops: 2 ACT table switches per j (each reload is
                1.3us)."""
                at_j = []
                for h in range(NH):
                    ph = h % 2
                    qrow = qt_g[h // 2][
                        ph * 64 : ph * 64 + 64, j * 512 : (j + 1) * 512
                    ]
                    # scoresT [256, 512] as one [128, 1024] tile
                    # (rc chunks in col halves); K=64 row-tiled pair
                    # with the other head of hp runs concurrently.
                    sc = ps_c.tile([128, 1024], f32, tag="sc", name=f"sc{j}_{h}")
                    for rc in range(2):
                        nc.tensor.matmul(
                            sc[:, rc * 512 : (rc + 1) * 512],
                            hslice(kpT_sb, h)[:, rc * 128 : (rc + 1) * 128],
                            qrow,
                            start=True,
                            stop=True,
                        )
                    a = p_at.tile([128, 1024], cdt, tag="at", name=f"at{j}_{h}")
                    nc.scalar.activation(
                        a, sc, mybir.ActivationFunctionType.Exp, scale=0.125
                    )
                    at_j.append(a)
                return at_j

            # software-pipelined over j: den(j)+recip(j) first (so the ACT
            # reciprocals chase the exps), then scores(j+1) interleave as
            # PE filler while ACT drains, then the dense PV/fin tail of j.
            at_j = emit_scores_exp(0)
            for j in range(NT):
                oT = [p_ot.tile([128, 512], cdt, tag="ot", name=f"oT{j}_{t}") for t in range(4)]
                # pass 1: denominators for all 4 pairs (needs only at_j) so
                # the ACT reciprocals run right after the exps -- ACT
                # stream is [exp x8, recip x2] per j (2 table switches).
                # Two pairs share one [128, 1024] den tile (one bank per
                # pair) -> one Reciprocal op covers both.
                recs = []
                for g in range(2):  # pair groups (hp = 2g, 2g+1)
                    bc = ps_c.tile([128, 1024], f32, tag="bc", bufs=1, name=f"bc{j}_{g}")
                    for q in range(2):
                        hp = g * 2 + q
                        ats = at_j[hp * 2 : hp * 2 + 2]
                        bank = bc[:, q * 512 : (q + 1) * 512]
                        # den broadcast via all-ones stationary matmuls over
                        # attnT: rows 0..63 = den_h0, rows 64..127 = den_h1
                        # (col-tiled concurrent).  A zero-matmul sets
                        # has_written for the whole bank so the den matmuls
                        # can accumulate with start=False (the bank-wide
                        # clear of start=True would race the col-tiled
                        # pair).
                        nc.tensor.matmul(
                            bank, zeros128, ats[0][:, 0:512],
                            start=True, stop=False, skip_group_check=True,
                        )
                        for hh in range(2):
                            for rc in range(2):
                                nc.tensor.matmul(
                                    bank[hh * 64 : (hh + 1) * 64, :],
                                    ones_blk,
                                    ats[hh][:, rc * 512 : (rc + 1) * 512],
                                    start=False,
                                    stop=(rc == 1),
                                    skip_group_check=True,
                                )
                    # evacuation doubles as the reciprocal: rec = 1/den
                    rec_sb = p_bc.tile([128, 1024], cdt, tag="bcs", name=f"rec{j}_{g}")
                    act_recip(rec_sb, bc)
                    recs.append(rec_sb)
                    if debug and j == 0 and g == 0:
                        dbg_sb = p_bc.tile([1, 4096], f32, tag="dbg", name="dbg_sb")
                        nc.scalar.copy(dbg_sb[0:1, 0:512], bc[0:1, 0:512])
                        nc.scalar.copy(dbg_sb[0:1, 512:1024], bc[64:65, 0:512])
                        nc.vector.tensor_copy(dbg_sb[0:1, 1024:1536], rec_sb[0:1, 0:512])
                        nc.vector.tensor_copy(dbg_sb[0:1, 1536:2048], rec_sb[64:65, 0:512])
                        nc.sync.dma_start(out=dbg_d, in_=dbg_sb)

                # prefetch next j's scores/exps: PE filler between this j's
                # den matmuls and its PV/fin tail; ACT sees them after this
                # j's reciprocals.
                at_next = emit_scores_exp(j + 1) if j + 1 < NT else None

                # pass 2: PV + normalize per pair
                for hp in range(4):
                    ats = at_j[hp * 2 : hp * 2 + 2]
                    # PV pair -> two [128, 512] PSUM tiles: head hh=0 at
                    # rows 0..63 of tile A, hh=1 at rows 64..127 of tile B
                    # (distinct col-groups AND banks -> concurrent on PE).
                    ops = [
                        ps_c.tile([128, 512], f32, tag="op", bufs=2, name=f"op{j}_{hp}_{hh}")
                        for hh in range(2)
                    ]
                    for hh in range(2):
                        h = hp * 2 + hh
                        for rc in range(2):
                            nc.tensor.matmul(
                                ops[hh][hh * 64 : hh * 64 + 64, :],
                                vext[:, h, rc, :],
                                ats[hh][:, rc * 512 : (rc + 1) * 512],
                                start=(rc == 0),
                                stop=(rc == 1),
                            )
                    for hh in range(2):
                        nc.vector.tensor_mul(
                            oT[hp][hh * 64 : (hh + 1) * 64, :],
                            ops[hh][hh * 64 : hh * 64 + 64, :],
                            recs[hp // 2][
                                hh * 64 : (hh + 1) * 64,
                                (hp % 2) * 512 : (hp % 2 + 1) * 512,
                            ],
                        )

                # y tiles: [128, 512] per n-subchunk; fp32 + bias via DVE
                for s in range(4):
                    fp = ps_c.tile([128, 512], f32, tag="op", bufs=2, name=f"fp{j}_{s}")
                    for dm in range(4):
                        nc.tensor.matmul(
                            fp,
                            oT[dm][:, s * 128 : (s + 1) * 128],
                            w_sb["wo"][:, dm, :],
                            start=(dm == 0),
                            stop=(dm == 3),
                        )
                    fin = p_fin.tile([128, 512], f32, tag="fin", name=f"fin_{j}_{s}")
                    nc.vector.tensor_add(fin, fp, bias_bc[:, 0, :])
                    nc.sync.dma_start(
                        out=y_r[:, j * 4 + s : j * 4 + s + 1, :], in_=fin.unsqueeze(1)
                    )
                at_j = at_next

    nc.compile()
    _built["nc"] = nc
    return nc


def prep_ef(E):
    """[NH, SEQ, R] -> [SEQ//128, 128, NH, R] bf16 (one contiguous block per
    128-row seq tile)."""
    np_c = ml_dtypes.bfloat16
    e = np.asarray(E).reshape(NH, SEQ // 128, 128, R)
    return np.ascontiguousarray(e.transpose(1, 2, 0, 3), dtype=np_c)


def _runner():
    """Build (once) a cached jitted 8-core executor for the Bass module."""
    if "run" in _built:
        return _built["run"]

    import jax
    import numpy as _np

    import concourse.mybir as mybir
    from concourse import bass2jax

    bass2jax.install_neuronx_cc_hook()
    nc = _build()

    part_name = nc.partition_id_tensor.name if nc.partition_id_tensor else None
    in_names, out_names, out_avals = [], [], []
    for alloc in nc.m.functions[0].allocations:
        if not isinstance(alloc, mybir.MemoryLocationSet):
            continue
        name = alloc.memorylocations[0].name
        if alloc.kind == "ExternalInput":
            if name != part_name:
                in_names.append(name)
        elif alloc.kind == "ExternalOutput":
            out_names.append(name)
            out_avals.append(
                jax.core.ShapedArray(
                    tuple(alloc.tensor_shape), mybir.dt.np(alloc.dtype)
                )
            )
    n_outs = len(out_avals)
    all_in_names = tuple(
        in_names + out_names + ([part_name] if part_name else [])
    )

    from jax.sharding import NamedSharding

    def _body(*args):
        operands = list(args)
        if part_name is not None:
            operands.append(bass2jax.partition_id_tensor())
        outs = bass2jax._bass_exec_p.bind(
            *operands,
            out_avals=tuple(out_avals),
            in_names=all_in_names,
            out_names=tuple(out_names),
            lowering_input_output_aliases=(),
            sim_require_finite=True,
            sim_require_nnan=True,
            nc=nc,
        )
        return tuple(outs)

    devices = jax.devices()[:NCORES]
    mesh = bass2jax.Mesh(_np.asarray(devices), ("core",))
    p_core = bass2jax.PartitionSpec("core")
    p_repl = bass2jax.PartitionSpec()
    # "x" is per-core; every other input is replicated across cores.
    # zero output buffers ride along as per-core params (hook requires params).
    in_specs = tuple(p_core if n == "x" else p_repl for n in in_names) + (
        p_core,
    ) * n_outs
    sharded = jax.jit(
        bass2jax.shard_map(
            _body,
            mesh=mesh,
            in_specs=in_specs,
            out_specs=(p_core,) * n_outs,
            check_rep=False,
        ),
        keep_unused=True,
    )
    sh_core = NamedSharding(mesh, p_core)
    sh_repl = NamedSharding(mesh, p_repl)
    dev_cache = {}

    zero_cache = {}

    def run(in_maps):
        args = []
        for name in in_names:
            if name == "x":
                xc = np.concatenate([np.asarray(m[name]) for m in in_maps], axis=0)
                args.append(jax.device_put(xc, sh_core))
            else:
                a = np.asarray(in_maps[0][name])
                key = (name, a.shape, str(a.dtype), hash(a.tobytes()))
                if key not in dev_cache:
                    dev_cache.clear() if len(dev_cache) > 64 else None
                    dev_cache[key] = jax.device_put(a, sh_repl)
                args.append(dev_cache[key])
        for i, a in enumerate(out_avals):
            if i not in zero_cache:
                zero_cache[i] = jax.device_put(
                    np.zeros((NCORES * a.shape[0], *a.shape[1:]), a.dtype), sh_core
                )
            args.append(zero_cache[i])
        out_arrs = sharded(*args)
        return [
            {
                name: np.asarray(out_arrs[i]).reshape(
                    NCORES, *out_avals[i].shape
                )[c]
                for i, name in enumerate(out_names)
            }
            for c in range(NCORES)
        ]

    _built["run"] = run
    return run


def make_in_maps(x, wq, wk, wv, E, F, w_out, b_out):
    """Full inputs -> list of per-core input dicts in kernel layouts."""
    np_c = ml_dtypes.bfloat16
    shared = {
        "wq": np.ascontiguousarray(wq, dtype=np_c),
        "wk": np.ascontiguousarray(wk, dtype=np_c),
        "wv": np.ascontiguousarray(wv, dtype=np_c),
        "E": prep_ef(E),
        "F": prep_ef(F),
        "w_out": np.ascontiguousarray(w_out, dtype=np_c),
        "b_out": np.ascontiguousarray(b_out, dtype=np.float32),
    }
    return [
        {
            "x": np.ascontiguousarray(np.asarray(x[i]).T, dtype=np_c),
            **shared,
        }
        for i in range(NCORES)
    ]


def kernel(x, wq, wk, wv, E, F, w_out, b_out):
    """Full inputs in, full output out. Shards batch across 8 cores."""
    run = _runner()
    in_maps = make_in_maps(x, wq, wk, wv, E, F, w_out, b_out)
    results = run(in_maps)
    return np.stack([results[i]["y"] for i in range(NCORES)], axis=0)


if __name__ == "__main__":
    xs = {
        "x": np.random.randn(BATCH, SEQ, DM).astype(np.float32),
        "wq": np.random.randn(DM, DM).astype(np.float32) * 0.05,
        "wk": np.random.randn(DM, DM).astype(np.float32) * 0.05,
        "wv": np.random.randn(DM, DM).astype(np.float32) * 0.05,
        "E": np.random.randn(NH, SEQ, R).astype(np.float32) * 0.03,
        "F": np.random.randn(NH, SEQ, R).astype(np.float32) * 0.03,
        "w_out": np.random.randn(DM, DM).astype(np.float32) * 0.05,
        "b_out": np.zeros(DM, np.float32),
    }
    y = kernel(**xs)
    print(y.shape, y.dtype)


# revision 38
# speedup vs baseline: 1.4877x; 1.0428x over previous
"""Linformer multi-head attention on 8 Trainium2 NeuronCores.

Sharding: data-parallel over batch (BATCH=8 -> 1 batch element per core).
Each core runs the full per-batch computation:
  q = x@wq, k = x@wk, v = x@wv            (per head h: 64-dim slices)
  k_proj[h] = E[h].T @ k[h]   [256, 64]   (contraction over seq)
  v_proj[h] = F[h].T @ v[h]   [256, 64]
  scores = q @ k_proj.T / 8   [4096, 256]
  attn = softmax(scores)  ;  out = attn @ v_proj
  y = concat_heads(out) @ w_out + b_out

v2 design notes (vs v1 at 506us):
  - x is transposed on HOST -> xT [512, 4096]; no on-chip transposes.
  - E/F are relayouted on HOST to [32 tiles, 128, 8 heads, 256] so each
    (j, s) DMA is one fully-contiguous 512KB block.
  - k_projT/v_projT accumulate in 4 persistent PSUM banks across all 32
    seq-tiles (no DVE partial adds).  A zero-matmul initializes each bank
    (has_written set everywhere) so every real matmul uses start=False --
    avoids the bank-wide has_written clear racing between interleaved
    accumulation regions.
  - M=64 kp/vp matmuls and K=64 score matmuls run as tile_position pairs
    (col/row-group concurrency, ~2x).
  - softmax denominator comes free from the PV matmul via an appended
    ones-column (row 64); per head-PAIR the two PV outputs land in one
    [128, 1024] PSUM tile so one reciprocal_approx_fast [1, 1024] handles
    both heads (v1 used full-precision reciprocal: 3.3us/op, 213us total).
  - reciprocal -> broadcast via rank-1 PE matmuls in float32r (full rate
    at N=512; plain f32 matmul is 4x slower).
  - evacuation work split between ScalarE (qt, bc) and VectorE (kv, oT
    muls, fin bias-adds); exp on ScalarE in [128, 1024] ops.

Compute dtype is bf16 (inputs cast on host) with fp32 PSUM accumulation.
"""

import os

import numpy as np
import ml_dtypes

BATCH, SEQ, DM = 8, 4096, 512
NH, DH, R = 8, 64, 256
NCORES = 8
NT = SEQ // 512  # 8 big n-tiles of 512 rows

_built = {}


def _build():
    """Build the Bass module (once per process)."""
    if "nc" in _built:
        return _built["nc"]

    from contextlib import ExitStack

    import concourse.bass as bass
    import concourse.bacc as bacc
    import concourse.mybir as mybir
    import concourse.tile as tile
    from concourse.masks import make_identity

    f32 = mybir.dt.float32
    f32r = mybir.dt.float32r
    cdt = mybir.dt.bfloat16

    nc = bacc.Bacc("TRN2", target_bir_lowering=False, debug=False)

    # xT: host-transposed [DM, SEQ]
    x_d = nc.dram_tensor("x", [DM, SEQ], cdt, kind="ExternalInput").ap()
    wq_d = nc.dram_tensor("wq", [DM, DM], cdt, kind="ExternalInput").ap()
    wk_d = nc.dram_tensor("wk", [DM, DM], cdt, kind="ExternalInput").ap()
    wv_d = nc.dram_tensor("wv", [DM, DM], cdt, kind="ExternalInput").ap()
    # E/F host layout: [ti, p, h, r] with ti = j*4+s, seq = ti*128+p
    e_d = nc.dram_tensor("E", [SEQ // 128, 128, NH, R], cdt, kind="ExternalInput").ap()
    f_d = nc.dram_tensor("F", [SEQ // 128, 128, NH, R], cdt, kind="ExternalInput").ap()
    wo_d = nc.dram_tensor("w_out", [DM, DM], cdt, kind="ExternalInput").ap()
    b_d = nc.dram_tensor("b_out", [DM], f32, kind="ExternalInput").ap()
    y_d = nc.dram_tensor("y", [SEQ, DM], f32, kind="ExternalOutput").ap()
    debug = os.environ.get("LINF_DEBUG", "0") == "1"
    if debug:
        dbg_d = nc.dram_tensor("dbg", [1, 4096], f32, kind="ExternalOutput").ap()

    with tile.TileContext(nc) as tc, ExitStack() as ctx:
        singles = ctx.enter_context(tc.tile_pool(name="singles", bufs=1))

        ident = singles.tile([128, 128], cdt)
        make_identity(nc, ident)
        ones_blk = singles.tile([128, 64], cdt)
        nc.vector.memset(ones_blk, 1.0)
        zeros128 = singles.tile([128, 128], cdt)
        nc.vector.memset(zeros128, 0.0)

        def act_recip(out, in_):
            """ACT Reciprocal LUT (bass blocks it for accuracy; softmax
            denominators only need ~1e-2 so the LUT is fine here)."""
            eng = nc.scalar
            ins = [eng.lower_ap(in_)]
            for val in (0.0, 1.0, 0.0):  # bias, scale, alpha
                ins.append(mybir.ImmediateValue(dtype=f32, value=val))
            return eng.add_instruction(
                mybir.InstActivation(
                    name=nc.get_next_instruction_name(),
                    func=mybir.ActivationFunctionType.Reciprocal,
                    ins=ins,
                    outs=[eng.lower_ap(out)],
                )
            )
        # bias replicated [128, 2, 512] for the [128, 1024] fin bias-add
        bias_bc = singles.tile([128, 2, DM], f32)
        b_bc_ap = bass.AP(
            tensor=b_d.tensor,
            offset=b_d.offset,
            ap=[[0, 128], [0, 2]] + list(b_d.ap),
        )
        nc.sync.dma_start(out=bias_bc, in_=b_bc_ap)

        # weights as [128, dk, 512]: chunk dk holds rows dk*128..+128
        w_sb = {}
        for name, d in (("wq", wq_d), ("wk", wk_d), ("wv", wv_d), ("wo", wo_d)):
            t = singles.tile([128, 4, DM], cdt, name=f"w_{name}")
            nc.sync.dma_start(out=t, in_=d.rearrange("(dk p) m -> p dk m", p=128))
            w_sb[name] = t

        # QT global [512, 4096] as 4 tiles [128, 4096]; tile t = heads 2t,2t+1
        qt_g = [singles.tile([128, SEQ], cdt, tag=f"qt{t}", name=f"qt{t}") for t in range(4)]
        # per-head low-rank projections, transposed [64, 256], packed 4/tile:
        # head h -> tile t=h//4, partition half ph=h%2, col half ch=(h//2)%2
        kpT_sb = [singles.tile([128, 2 * R], cdt, tag=f"kp{t}", name=f"kpT{t}") for t in range(2)]
        vpT_sb = [singles.tile([128, 2 * R], cdt, tag=f"vp{t}", name=f"vpT{t}") for t in range(2)]

        def hslice(sb, h):
            """[64, 256] slice of packed kpT/vpT for head h."""
            t, ph, ch = h // 4, h % 2, (h // 2) % 2
            return sb[t][ph * 64 : (ph + 1) * 64, ch * R : (ch + 1) * R]

        # v_proj natural chunks: [128, 2, 64] per head
        vext = singles.tile([128, NH, 2, 64], cdt)

        # ---------------- Phase AB: QT, k_projT, v_projT ----------------
        with (
            tc.tile_pool(name="p_x", bufs=3) as p_x,
            tc.tile_pool(name="p_ef", bufs=3) as p_ef,
            tc.tile_pool(name="p_kv", bufs=6) as p_kv,
            tc.tile_pool(name="ps_acc", bufs=1, space="PSUM") as ps_acc,
            tc.tile_pool(name="ps_mm", bufs=4, space="PSUM") as ps_mm,
        ):
            # persistent PSUM accumulators: 4 banks, live all of phase AB
            kpT_ps = [
                ps_acc.tile([128, 2 * R], f32, tag=f"kpp{t}", name=f"kpT_ps{t}")
                for t in range(2)
            ]
            vpT_ps = [
                ps_acc.tile([128, 2 * R], f32, tag=f"vpp{t}", name=f"vpT_ps{t}")
                for t in range(2)
            ]
            # init: one full-bank zero-matmul (lhsT=0 so rhs content is
            # irrelevant) sets has_written on every element so all real
            # accumulation matmuls can use start=False (see header).
            for t in range(2):
                for acc in (kpT_ps[t], vpT_ps[t]):
                    nc.tensor.matmul(
                        acc,
                        zeros128,
                        w_sb["wq"][:, 0, :],
                        start=True,
                        stop=False,
                        skip_group_check=True,
                    )

            x_r = x_d.rearrange("(dk p) n -> p dk n", p=128)
            e_r = e_d.rearrange("t p h r -> p t h r")
            f_r = f_d.rearrange("t p h r -> p t h r")
            for j in range(NT):  # 8 n-tiles of 512 rows
                xt = p_x.tile([128, 4, 512], cdt, tag="xt", name=f"xt_{j}")
                nc.sync.dma_start(out=xt, in_=x_r[:, :, j * 512 : (j + 1) * 512])

                e_t = p_ef.tile([128, 4, NH, R], cdt, tag="ef_e", name=f"e_{j}")
                f_t = p_ef.tile([128, 4, NH, R], cdt, tag="ef_f", name=f"f_{j}")
                nc.sync.dma_start(out=e_t, in_=e_r[:, j * 4 : (j + 1) * 4, :, :])
                nc.sync.dma_start(out=f_t, in_=f_r[:, j * 4 : (j + 1) * 4, :, :])
                e_ts = [e_t[:, s, :, :] for s in range(4)]
                f_ts = [f_t[:, s, :, :] for s in range(4)]

                # QT_j[dq] [128, 512] = sum_dk wq[dk, dq-chunk].T-form @ xT[dk]
                for dq in range(4):
                    pq = ps_mm.tile([128, 512], f32, tag="pmm", name=f"pq_{j}_{dq}")
                    for dk in range(4):
                        nc.tensor.matmul(
                            pq,
                            w_sb["wq"][:, dk, dq * 128 : (dq + 1) * 128],
                            xt[:, dk, :],
                            start=(dk == 0),
                            stop=(dk == 3),
                        )
                    nc.scalar.copy(qt_g[dq][:, j * 512 : (j + 1) * 512], pq)

                # K/V per 128-row subtile s, then accumulate projections
                for s in range(4):
                    for wname, ef in (("wk", e_ts[s]), ("wv", f_ts[s])):
                        pk = ps_mm.tile([128, 512], f32, tag="pmm", name=f"pk_{j}_{s}")
                        for dk in range(4):
                            nc.tensor.matmul(
                                pk,
                                xt[:, dk, s * 128 : (s + 1) * 128],
                                w_sb[wname][:, dk, :],
                                start=(dk == 0),
                                stop=(dk == 3),
                            )
                        kv_sb = p_kv.tile([128, 512], cdt, tag="kv", name=f"kv_{j}_{s}")
                        nc.vector.tensor_copy(kv_sb, pk)

                        acc = kpT_ps if wname == "wk" else vpT_ps
                        last = (j == NT - 1) and (s == 3)
                        # col-tiled pairs: heads (2i, 2i+1) -> partition
                        # halves 0/64 of the same bank, concurrent on PE.
                        for h in range(NH):
                            t, ph, ch = h // 4, h % 2, (h // 2) % 2
                            nc.tensor.matmul(
                                acc[t][
                                    ph * 64 : (ph + 1) * 64,
                                    ch * R : (ch + 1) * R,
                                ],
                                kv_sb[:, h * 64 : (h + 1) * 64],
                                ef[:, h, :],
                                start=False,
                                stop=last,
                                skip_group_check=True,
                            )

            for t in range(2):
                nc.scalar.copy(kpT_sb[t], kpT_ps[t])
                nc.scalar.copy(vpT_sb[t], vpT_ps[t])

        # ---------------- Phase C: attention + output dense ----------------
        y_r = y_d.rearrange("(t p) m -> p t m", p=128)  # t = j*4+s
        with (
            tc.tile_pool(name="p_at", bufs=18) as p_at,
            tc.tile_pool(name="p_bc", bufs=6) as p_bc,
            tc.tile_pool(name="p_ot", bufs=8) as p_ot,
            tc.tile_pool(name="p_fin", bufs=4) as p_fin,
            tc.tile_pool(name="ps_c", bufs=2, space="PSUM") as ps_c,
        ):
            # build vext: transpose v_projT[h] chunks to natural
            for h in range(NH):
                pv = ps_c.tile([128, 512], cdt, tag="op", bufs=4, name="pv")
                for rc in range(2):
                    nc.tensor.transpose(
                        pv[:, rc * 64 : (rc + 1) * 64],
                        hslice(vpT_sb, h)[:, rc * 128 : (rc + 1) * 128],
                        ident[(h % 2) * 64 : (h % 2) * 64 + 64, (h % 2) * 64 : (h % 2) * 64 + 64],
                    )
                for rc in range(2):
                    nc.vector.tensor_copy(
                        vext[:, h, rc, :], pv[:, rc * 64 : (rc + 1) * 64]
                    )

            def emit_scores_exp(j):
                """Scores + exp for all 8 heads of tile j.  Batched so the
                ACT stream runs the 8 Exp ops back-to-back, then the 2
                Reciprocal ops: 2 ACT table switches per j (each reload is
                1.3us)."""
                at_j = []
                for h in range(NH):
                    ph = h % 2
                    qrow = qt_g[h // 2][
                        ph * 64 : ph * 64 + 64, j * 512 : (j + 1) * 512
                    ]
                    # scoresT [256, 512] as one [128, 1024] tile
                    # (rc chunks in col halves); K=64 row-tiled pair
                    # with the other head of hp runs concurrently.
                    sc = ps_c.tile([128, 1024], f32, tag="sc", name=f"sc{j}_{h}")
                    for rc in range(2):
                        nc.tensor.matmul(
                            sc[:, rc * 512 : (rc + 1) * 512],
                            hslice(kpT_sb, h)[:, rc * 128 : (rc + 1) * 128],
                            qrow,
                            start=True,
                            stop=True,
                        )
                    a = p_at.tile([128, 1024], cdt, tag="at", name=f"at{j}_{h}")
                    nc.scalar.activation(
                        a, sc, mybir.ActivationFunctionType.Exp, scale=0.125
                    )
                    at_j.append(a)
                return at_j

            # software-pipelined over j: den(j)+recip(j) first (so the ACT
            # reciprocals chase the exps), then scores(j+1) interleave as
            # PE filler while ACT drains, then the dense PV/fin tail of j.
            at_j = emit_scores_exp(0)
            for j in range(NT):
                oT = [p_ot.tile([128, 512], cdt, tag="ot", name=f"oT{j}_{t}") for t in range(4)]
                # pass 1: denominators for all 4 pairs (needs only at_j) so
                # the ACT reciprocals run right after the exps -- ACT
                # stream is [exp x8, recip x2] per j (2 table switches).
                # Two pairs share one [128, 1024] den tile (one bank per
                # pair) -> one Reciprocal op covers both.
                recs = []
                bcs = []
                for hp in range(4):
                    ats = at_j[hp * 2 : hp * 2 + 2]
                    # den broadcast via all-ones stationary matmuls over
                    # attnT: rows 0..63 = den_h0, rows 64..127 = den_h1
                    # (col-tiled concurrent).  A zero-matmul sets
                    # has_written for the whole bank so the den matmuls
                    # can accumulate with start=False (the bank-wide
                    # clear of start=True would race the col-tiled pair).
                    bc = ps_c.tile([128, 512], f32, tag="op", bufs=4, name=f"bc{j}_{hp}")
                    bcs.append(bc)
                    nc.tensor.matmul(
                        bc, zeros128, ats[0][:, 0:512],
                        start=True, stop=False, skip_group_check=True,
                    )
                    for hh in range(2):
                        for rc in range(2):
                            nc.tensor.matmul(
                                bc[hh * 64 : (hh + 1) * 64, :],
                                ones_blk,
                                ats[hh][:, rc * 512 : (rc + 1) * 512],
                                start=False,
                                stop=(rc == 1),
                                skip_group_check=True,
                            )
                # all reciprocals adjacent so the ACT stream keeps them in
                # one table-switch window
                for hp in range(4):
                    # evacuation doubles as the reciprocal: rec = 1/den
                    rec_sb = p_bc.tile([128, 512], cdt, tag="bcs", name=f"rec{j}_{hp}")
                    act_recip(rec_sb, bcs[hp])
                    recs.append(rec_sb)
                if debug and j == 0:
                    dbg_sb = p_bc.tile([1, 4096], f32, tag="dbg", name="dbg_sb")
                    nc.scalar.copy(dbg_sb[0:1, 0:512], bcs[0][0:1, :])
                    nc.scalar.copy(dbg_sb[0:1, 512:1024], bcs[0][64:65, :])
                    nc.vector.tensor_copy(dbg_sb[0:1, 1024:1536], recs[0][0:1, :])
                    nc.vector.tensor_copy(dbg_sb[0:1, 1536:2048], recs[0][64:65, :])
                    nc.sync.dma_start(out=dbg_d, in_=dbg_sb)

                # prefetch next j's scores/exps: PE filler between this j's
                # den matmuls and its PV/fin tail; ACT sees them after this
                # j's reciprocals.
                at_next = emit_scores_exp(j + 1) if j + 1 < NT else None

                # pass 2: PV + normalize per pair
                for hp in range(4):
                    ats = at_j[hp * 2 : hp * 2 + 2]
                    # PV pair -> two [128, 512] PSUM tiles: head hh=0 at
                    # rows 0..63 of tile A, hh=1 at rows 64..127 of tile B
                    # (distinct col-groups AND banks -> concurrent on PE).
                    ops = [
                        ps_c.tile([128, 512], f32, tag="op", bufs=4, name=f"op{j}_{hp}_{hh}")
                        for hh in range(2)
                    ]
                    for hh in range(2):
                        h = hp * 2 + hh
                        for rc in range(2):
                            nc.tensor.matmul(
                                ops[hh][hh * 64 : hh * 64 + 64, :],
                                vext[:, h, rc, :],
                                ats[hh][:, rc * 512 : (rc + 1) * 512],
                                start=(rc == 0),
                                stop=(rc == 1),
                            )
                    for hh in range(2):
                        nc.vector.tensor_mul(
                            oT[hp][hh * 64 : (hh + 1) * 64, :],
                            ops[hh][hh * 64 : hh * 64 + 64, :],
                            recs[hp][hh * 64 : (hh + 1) * 64, :],
                        )

                # y tiles: [128, 512] per n-subchunk; fp32 + bias via DVE
                for s in range(4):
                    fp = ps_c.tile([128, 512], f32, tag="op", bufs=4, name=f"fp{j}_{s}")
                    for dm in range(4):
                        nc.tensor.matmul(
                            fp,
                            oT[dm][:, s * 128 : (s + 1) * 128],
                            w_sb["wo"][:, dm, :],
                            start=(dm == 0),
                            stop=(dm == 3),
                        )
                    fin = p_fin.tile([128, 512], f32, tag="fin", name=f"fin_{j}_{s}")
                    nc.vector.tensor_add(fin, fp, bias_bc[:, 0, :])
                    nc.sync.dma_start(
                        out=y_r[:, j * 4 + s : j * 4 + s + 1, :], in_=fin.unsqueeze(1)
                    )
                at_j = at_next

    nc.compile()
    _built["nc"] = nc
    return nc


def prep_ef(E):
    """[NH, SEQ, R] -> [SEQ//128, 128, NH, R] bf16 (one contiguous block per
    128-row seq tile)."""
    np_c = ml_dtypes.bfloat16
    e = np.asarray(E).reshape(NH, SEQ // 128, 128, R)
    return np.ascontiguousarray(e.transpose(1, 2, 0, 3), dtype=np_c)


def _runner():
    """Build (once) a cached jitted 8-core executor for the Bass module."""
    if "run" in _built:
        return _built["run"]

    import jax
    import numpy as _np

    import concourse.mybir as mybir
    from concourse import bass2jax

    bass2jax.install_neuronx_cc_hook()
    nc = _build()

    part_name = nc.partition_id_tensor.name if nc.partition_id_tensor else None
    in_names, out_names, out_avals = [], [], []
    for alloc in nc.m.functions[0].allocations:
        if not isinstance(alloc, mybir.MemoryLocationSet):
            continue
        name = alloc.memorylocations[0].name
        if alloc.kind == "ExternalInput":
            if name != part_name:
                in_names.append(name)
        elif alloc.kind == "ExternalOutput":
            out_names.append(name)
            out_avals.append(
                jax.core.ShapedArray(
                    tuple(alloc.tensor_shape), mybir.dt.np(alloc.dtype)
                )
            )
    n_outs = len(out_avals)
    all_in_names = tuple(
        in_names + out_names + ([part_name] if part_name else [])
    )

    from jax.sharding import NamedSharding

    def _body(*args):
        operands = list(args)
        if part_name is not None:
            operands.append(bass2jax.partition_id_tensor())
        outs = bass2jax._bass_exec_p.bind(
            *operands,
            out_avals=tuple(out_avals),
            in_names=all_in_names,
            out_names=tuple(out_names),
            lowering_input_output_aliases=(),
            sim_require_finite=True,
            sim_require_nnan=True,
            nc=nc,
        )
        return tuple(outs)

    devices = jax.devices()[:NCORES]
    mesh = bass2jax.Mesh(_np.asarray(devices), ("core",))
    p_core = bass2jax.PartitionSpec("core")
    p_repl = bass2jax.PartitionSpec()
    # "x" is per-core; every other input is replicated across cores.
    # zero output buffers ride along as per-core params (hook requires params).
    in_specs = tuple(p_core if n == "x" else p_repl for n in in_names) + (
        p_core,
    ) * n_outs
    sharded = jax.jit(
        bass2jax.shard_map(
            _body,
            mesh=mesh,
            in_specs=in_specs,
            out_specs=(p_core,) * n_outs,
            check_rep=False,
        ),
        keep_unused=True,
    )
    sh_core = NamedSharding(mesh, p_core)
    sh_repl = NamedSharding(mesh, p_repl)
    dev_cache = {}

    zero_cache = {}

    def run(in_maps):
        args = []
        for name in in_names:
            if name == "x":
                xc = np.concatenate([np.asarray(m[name]) for m in in_maps], axis=0)
                args.append(jax.device_put(xc, sh_core))
            else:
                a = np.asarray(in_maps[0][name])
                key = (name, a.shape, str(a.dtype), hash(a.tobytes()))
                if key not in dev_cache:
                    dev_cache.clear() if len(dev_cache) > 64 else None
                    dev_cache[key] = jax.device_put(a, sh_repl)
                args.append(dev_cache[key])
        for i, a in enumerate(out_avals):
            if i not in zero_cache:
                zero_cache[i] = jax.device_put(
                    np.zeros((NCORES * a.shape[0], *a.shape[1:]), a.dtype), sh_core
                )
            args.append(zero_cache[i])
        out_arrs = sharded(*args)
        return [
            {
                name: np.asarray(out_arrs[i]).reshape(
                    NCORES, *out_avals[i].shape
                )[c]
                for i, name in enumerate(out_names)
            }
            for c in range(NCORES)
        ]

    _built["run"] = run
    return run


def make_in_maps(x, wq, wk, wv, E, F, w_out, b_out):
    """Full inputs -> list of per-core input dicts in kernel layouts."""
    np_c = ml_dtypes.bfloat16
    shared = {
        "wq": np.ascontiguousarray(wq, dtype=np_c),
        "wk": np.ascontiguousarray(wk, dtype=np_c),
        "wv": np.ascontiguousarray(wv, dtype=np_c),
        "E": prep_ef(E),
        "F": prep_ef(F),
        "w_out": np.ascontiguousarray(w_out, dtype=np_c),
        "b_out": np.ascontiguousarray(b_out, dtype=np.float32),
    }
    return [
        {
            "x": np.ascontiguousarray(np.asarray(x[i]).T, dtype=np_c),
            **shared,
        }
        for i in range(NCORES)
    ]


def kernel(x, wq, wk, wv, E, F, w_out, b_out):
    """Full inputs in, full output out. Shards batch across 8 cores."""
    run = _runner()
    in_maps = make_in_maps(x, wq, wk, wv, E, F, w_out, b_out)
    results = run(in_maps)
    return np.stack([results[i]["y"] for i in range(NCORES)], axis=0)


if __name__ == "__main__":
    xs = {
        "x": np.random.randn(BATCH, SEQ, DM).astype(np.float32),
        "wq": np.random.randn(DM, DM).astype(np.float32) * 0.05,
        "wk": np.random.randn(DM, DM).astype(np.float32) * 0.05,
        "wv": np.random.randn(DM, DM).astype(np.float32) * 0.05,
        "E": np.random.randn(NH, SEQ, R).astype(np.float32) * 0.03,
        "F": np.random.randn(NH, SEQ, R).astype(np.float32) * 0.03,
        "w_out": np.random.randn(DM, DM).astype(np.float32) * 0.05,
        "b_out": np.zeros(DM, np.float32),
    }
    y = kernel(**xs)
    print(y.shape, y.dtype)
